# revision 1
# baseline (speedup 1.0000x reference)
"""ChebConv (K=5) distributed Trainium2 kernel over 8 NeuronCores.

Strategy: shard V across the 8 cores (row slices). Per Chebyshev step the
spmm is computed per-core for its row slice: neighbor features are fetched
with GPSIMD dma_gather (1KB elements = all 4 batches x 128 feat in bf16)
from a replicated full-V DRAM copy; the per-edge scale + segment-sum runs
on the TensorEngine as selection matmuls (S.T @ G accumulated in PSUM per
128-row block); an 8-way AllGather rebuilds the replicated x for the next
step. The final dense matmul (sum_k xk @ Wk) is local per core.
"""
import os
import numpy as np
import ml_dtypes

import concourse.bass as bass
import concourse.bacc as bacc
import concourse.mybir as mybir
import concourse.tile as tile
from concourse.bass_utils import run_bass_kernel_spmd

bf16 = ml_dtypes.bfloat16
P = 128


class Cfg:
    def __init__(self, V=50000, B=4, FIN=128, FOUT=128, K=5, NCORE=8):
        self.V, self.B, self.FIN, self.FOUT, self.K, self.NCORE = V, B, FIN, FOUT, K, NCORE
        self.C = B * FIN                      # 512 feature columns
        vpad = -(-V // (NCORE * P)) * (NCORE * P)
        self.VPAD = vpad                      # 50176
        self.VSLICE = vpad // NCORE           # 6272
        self.NBLK = self.VSLICE // P          # 49 row blocks per core
        # pieces: split each core's shard rows into two block-aligned pieces.
        # Piece buffers concat the piece across cores -> gather idx stays
        # within int16 (max NCORE*S1).
        self.PBLK = [self.NBLK // 2, self.NBLK - self.NBLK // 2]
        self.S = [self.PBLK[0] * P, self.PBLK[1] * P]      # rows per piece
        self.PSZ = [self.NCORE * self.S[0], self.NCORE * self.S[1]]

    def piece_idx(self, v):
        """Map node ids to (piece, index-in-piece-buffer). v: np.ndarray."""
        core = v // self.VSLICE
        r = v % self.VSLICE
        p = (r >= self.S[0]).astype(np.int64)
        idx = np.where(p == 0, core * self.S[0] + r,
                       core * self.S[1] + (r - self.S[0]))
        return p, idx


def preprocess(cfg, rows, cols, vals):
    """Sort edges by row, build the (uniform-across-cores) tile schedule and
    per-core packed gather/S-build arrays.

    Returns (schedule, percore) where schedule is a list over row blocks of
    (T_lo, T_hi) tile counts, and percore is a list of dicts with:
      gidx   [16, NTT*8] int16   wrapped gather indices (per tile of 128 edges)
      rowrel [128, NTT]   bf16   row - blockstart per edge lane
      vals1  [128, NTT]   bf16   edge values
      vals2  [128, NTT]   bf16   2x edge values
    """
    order = np.argsort(rows, kind="stable")
    r, c, v = rows[order], cols[order], vals[order]
    percore_groups = []
    for ci in range(cfg.NCORE):
        r0, r1 = ci * cfg.VSLICE, (ci + 1) * cfg.VSLICE
        lo_i, hi_i = np.searchsorted(r, [r0, r1])
        rc, cc, vc = r[lo_i:hi_i], c[lo_i:hi_i], v[lo_i:hi_i]
        blk = (rc - r0) // P
        bstart = np.searchsorted(blk, np.arange(cfg.NBLK))
        bend = np.searchsorted(blk, np.arange(cfg.NBLK) + 1)
        cp, cidx = cfg.piece_idx(cc)
        groups = []
        for b in range(cfg.NBLK):
            sl = slice(bstart[b], bend[b])
            eb_p, eb_i = cp[sl], cidx[sl]
            eb_r = rc[sl] - r0 - b * P
            eb_v = vc[sl]
            lo = eb_p == 0
            grp = []
            for m in (lo, ~lo):
                gi, gr, gv = eb_i[m], eb_r[m], eb_v[m]
                if not os.environ.get("KERNEL_NOSORT"):
                    o = np.argsort(gi, kind="stable")  # ascending addresses per gather
                    gi, gr, gv = gi[o], gr[o], gv[o]
                grp.append((gi, gr, gv))
            groups.append(tuple(grp))
        percore_groups.append(groups)

    # uniform schedule: per (block, half) tile count = max over cores
    schedule = []
    for b in range(cfg.NBLK):
        tlo = max(max(1, -(-len(g[b][0][0]) // P)) for g in percore_groups)
        thi = max(max(1, -(-len(g[b][1][0]) // P)) for g in percore_groups)
        schedule.append((tlo, thi))

    ntt = sum(tlo + thi for tlo, thi in schedule)
    percore = []
    for ci in range(cfg.NCORE):
        gidx = np.zeros((16, ntt * 8), np.int16)
        rowrel = np.zeros((P, ntt), bf16)
        vals1 = np.zeros((P, ntt), np.float32)
        t0 = 0
        for b in range(cfg.NBLK):
            for half in (0, 1):
                hc, hr, hv = percore_groups[ci][b][half]
                T = schedule[b][half]
                n = T * P
                ci_pad = np.zeros(n, np.int16)
                ci_pad[:len(hc)] = hc.astype(np.int16)
                # wrapped layout: idx[p, s] = edge[s*16 + p]
                gidx[:, t0 * 8:(t0 + T) * 8] = ci_pad.reshape(-1, 16).T
                rr = np.zeros(n, np.float32)
                rr[:len(hr)] = hr
                vv = np.zeros(n, np.float32)
                vv[:len(hv)] = hv
                rowrel[:, t0:t0 + T] = rr.reshape(T, P).T.astype(bf16)
                vals1[:, t0:t0 + T] = vv.reshape(T, P).T
                t0 += T
        percore.append(dict(
            gidx=np.tile(gidx, (8, 1)),   # idx AP spans 128 partitions; HW reads rows 0-15
            rowrel=rowrel,
            vals1=vals1.astype(bf16),
            vals2=(2.0 * vals1).astype(bf16),
        ))
    return schedule, ntt, percore


def build_graph(cfg, schedule, ntt, n_steps=None, with_coll=True, with_final=True,
                skip_gather=False, skip_mm=False):
    """Build the SPMD bass graph (identical for all cores)."""
    nc = bacc.Bacc()
    f32, bf, i16 = mybir.dt.float32, mybir.dt.bfloat16, mybir.dt.int16
    C, K, NBLK, VSLICE = cfg.C, cfg.K, cfg.NBLK, cfg.VSLICE

    # ---- parameters -----------------------------------------------------
    x0_p0 = nc.declare_dram_parameter("x0_p0", [cfg.PSZ[0], C], bf, isOutput=False)
    x0_p1 = nc.declare_dram_parameter("x0_p1", [cfg.PSZ[1], C], bf, isOutput=False)
    x0_own = nc.declare_dram_parameter("x0_own", [VSLICE, C], bf, isOutput=False)
    gidx_p = nc.declare_dram_parameter("gidx", [P, ntt * 8], i16, isOutput=False)
    rowrel_p = nc.declare_dram_parameter("rowrel", [P, ntt], bf, isOutput=False)
    vals1_p = nc.declare_dram_parameter("vals1", [P, ntt], bf, isOutput=False)
    vals2_p = nc.declare_dram_parameter("vals2", [P, ntt], bf, isOutput=False)
    iota4_p = nc.declare_dram_parameter("iota4", [P, 4 * P], bf, isOutput=False)
    wT_p = nc.declare_dram_parameter("wT", [P, K * P], bf, isOutput=False)
    biasrep_p = nc.declare_dram_parameter("biasrep", [P, P], f32, isOutput=False)
    out_p = nc.declare_dram_parameter("out", [cfg.B, VSLICE, cfg.FOUT], f32, isOutput=True)

    # ---- internal DRAM --------------------------------------------------
    # xk_own[k]: this core's rows of x_k (k=1..K-1); x_0 comes via x0_own.
    xk_own = [None] + [nc.dram_tensor(f"xk_own{k}", [VSLICE, C], bf) for k in range(1, K)]
    # xp[s][piece]: replicated per-piece gather sources for step s (s=1..K-2).
    xp = [None] + [
        [nc.dram_tensor(f"xp{s}_{pc}", [cfg.PSZ[pc], C], bf, addr_space="Shared")
         for pc in range(2)]
        for s in range(1, K - 1)
    ]

    replica_groups = [list(range(cfg.NCORE))]

    with tile.TileContext(nc) as tc:
        with (
            tc.tile_pool(name="const", bufs=1) as constp,
            tc.tile_pool(name="gbuf", bufs=2) as gbufp,
            tc.tile_pool(name="ltp", bufs=1) as ltp,
            tc.tile_pool(name="sbuf", bufs=4) as sbufp,
            tc.tile_pool(name="spool", bufs=8) as spoolp,
            tc.tile_pool(name="psum", bufs=3, space="PSUM") as psump,
        ):
            # resident constants
            gidx_t = constp.tile([P, ntt * 8], i16)
            nc.sync.dma_start(out=gidx_t[:], in_=gidx_p[:])
            rowrel_t = constp.tile([P, ntt], bf)
            nc.sync.dma_start(out=rowrel_t[:], in_=rowrel_p[:])
            vals1_t = constp.tile([P, ntt], bf)
            nc.sync.dma_start(out=vals1_t[:], in_=vals1_p[:])
            vals2_t = constp.tile([P, ntt], bf)
            nc.sync.dma_start(out=vals2_t[:], in_=vals2_p[:])
            iota4_t = constp.tile([P, 4 * P], bf)
            nc.sync.dma_start(out=iota4_t[:], in_=iota4_p[:])
            wT_t = constp.tile([P, K * P], bf)
            nc.sync.dma_start(out=wT_t[:], in_=wT_p[:])
            biasrep_t = constp.tile([P, P], f32)
            nc.sync.dma_start(out=biasrep_t[:], in_=biasrep_p[:])

            maxT = max(tlo + thi for tlo, thi in schedule)

            # ---- Chebyshev recurrence: spmm steps s=0..K-2 --------------
            for s in range(K - 1 if n_steps is None else n_steps):
                if s == 0:
                    src_lo, src_hi = x0_p0[:], x0_p1[:]
                else:
                    src_lo, src_hi = xp[s][0][:], xp[s][1][:]
                vals_t = vals1_t if s == 0 else vals2_t
                t0 = 0
                for b in range(NBLK):
                    tlo, thi = schedule[b]
                    T = tlo + thi
                    g_t = gbufp.tile([P, maxT, C], bf, tag="g")
                    if skip_gather:
                        nc.vector.memset(g_t[:], 0.0)
                    else:
                        for half, (toff, tcnt) in enumerate(((0, tlo), (tlo, thi))):
                            src = src_hi if half else src_lo
                            n = tcnt * P
                            nc.gpsimd.dma_gather(
                                out_ap=g_t[:, toff:toff + tcnt, :],
                                in_ap=src,
                                idxs_ap=gidx_t[:, (t0 + toff) * 8:(t0 + toff + tcnt) * 8],
                                num_idxs=n,
                                num_idxs_reg=n,
                                elem_size=C,
                                single_packet=False,
                            )
                    if skip_mm:
                        t0 += T
                        continue
                    psum_t = psump.tile([P, C], f32, tag="ps")
                    # selection matmuls: build S in groups of <=4 tiles
                    for g0 in range(0, T, 4):
                        gw = min(4, T - g0)
                        s_t = spoolp.tile([P, 4 * P], bf, tag="s")
                        rr = rowrel_t[:, t0 + g0:t0 + g0 + gw]
                        nc.vector.tensor_tensor(
                            out=s_t[:, :gw * P],
                            in0=rr.to_broadcast([P, gw, P]),
                            in1=iota4_t[:, :gw * P],
                            op=mybir.AluOpType.is_equal,
                        )
                        vv = vals_t[:, t0 + g0:t0 + g0 + gw]
                        nc.vector.tensor_tensor(
                            out=s_t[:, :gw * P],
                            in0=s_t[:, :gw * P].rearrange("p (g q) -> p g q", q=P),
                            in1=vv.to_broadcast([P, gw, P]),
                            op=mybir.AluOpType.mult,
                        )
                        for ti in range(gw):
                            t = g0 + ti
                            nc.tensor.matmul(
                                psum_t[:],
                                lhsT=s_t[:, ti * P:(ti + 1) * P],
                                rhs=g_t[:, t, :],
                                start=(t == 0),
                                stop=(t == T - 1),
                            )
                    # drain: x_{s+1} = psum (s==0, vals1) or psum - x_{s-1} (vals2)
                    xk_t = sbufp.tile([P, C], bf, tag="xk")
                    if s == 0:
                        nc.scalar.copy(out=xk_t[:], in_=psum_t[:])
                    else:
                        xprev_src = x0_own if s == 1 else xk_own[s - 1]
                        xprev_t = sbufp.tile([P, C], bf, tag="xprev")
                        nc.sync.dma_start(
                            out=xprev_t[:], in_=xprev_src[b * P:(b + 1) * P, :])
                        xprev_f = sbufp.tile([P, C], f32, tag="xprevf")
                        nc.scalar.copy(out=xprev_f[:], in_=xprev_t[:])
                        nc.vector.tensor_tensor(
                            out=xk_t[:], in0=psum_t[:], in1=xprev_f[:],
                            op=mybir.AluOpType.subtract,
                        )
                    nc.sync.dma_start(
                        out=xk_own[s + 1][b * P:(b + 1) * P, :], in_=xk_t[:])
                    t0 += T
                    # replicate finished pieces of x_{s+1} for the next step
                    if s < K - 2 and with_coll:
                        if b == cfg.PBLK[0] - 1:
                            nc.gpsimd.collective_compute(
                                "AllGather", mybir.AluOpType.bypass,
                                replica_groups=replica_groups,
                                ins=[xk_own[s + 1][:cfg.S[0], :]],
                                outs=[xp[s + 1][0][:]],
                            )
                        elif b == NBLK - 1:
                            nc.gpsimd.collective_compute(
                                "AllGather", mybir.AluOpType.bypass,
                                replica_groups=replica_groups,
                                ins=[xk_own[s + 1][cfg.S[0]:, :]],
                                outs=[xp[s + 1][1][:]],
                            )

            # ---- final dense matmul: out[b] = sum_k xk @ Weff_k + bias --
            # bulk-transpose each x_k batch column block [VSLICE,128] ->
            # [128, VSLICE] once, then 5 matmuls per 128-row tile.
            half_blks = [(0, (NBLK + 1) // 2), ((NBLK + 1) // 2, NBLK)]
            for bb in range(cfg.B if with_final else 0):
                for hb0, hb1 in half_blks:
                    if hb1 <= hb0:
                        continue
                    r0, r1 = hb0 * P, hb1 * P
                    lts = []
                    for k in range(K):
                        src = x0_own if k == 0 else xk_own[k]
                        lt = ltp.tile([P, (half_blks[0][1]) * P], bf, tag=f"lt{k}")
                        nc.sync.dma_start(
                            out=lt[:, :r1 - r0],
                            in_=src[r0:r1, bb * P:(bb + 1) * P],
                            transpose=True,
                        )
                        lts.append(lt)
                    for rt in range(hb0, hb1):
                        ro = (rt - hb0) * P
                        po = psump.tile([P, P], f32, tag="po")
                        for k in range(K):
                            nc.tensor.matmul(
                                po[:], lhsT=lts[k][:, ro:ro + P],
                                rhs=wT_t[:, k * P:(k + 1) * P],
                                start=(k == 0), stop=(k == K - 1),
                            )
                        ot = sbufp.tile([P, P], f32, tag="ot")
                        nc.vector.tensor_tensor(
                            out=ot[:], in0=po[:], in1=biasrep_t[:],
                            op=mybir.AluOpType.add,
                        )
                        nc.sync.dma_start(
                            out=out_p[bb, rt * P:(rt + 1) * P, :], in_=ot[:])
    return nc


def make_inputs_maps(cfg, schedule, ntt, percore, inputs, weight, bias):
    """Build per-core in_maps (host-side sharding + packing)."""
    V, C, K = cfg.V, cfg.C, cfg.K
    x0 = np.zeros((cfg.VPAD, C), np.float32)
    x0[:V] = np.transpose(inputs, (1, 0, 2)).reshape(V, C)
    x0 = x0.astype(bf16)

    # reference pairs xk[..., f*K+k] with weight.reshape(Fin*K, Fout)[f*K+k]
    wr = weight.reshape(K * cfg.FIN, cfg.FOUT)
    weff = np.stack([wr[np.arange(cfg.FIN) * K + k] for k in range(K)], 0)
    wT = np.concatenate([weff[k] for k in range(K)], axis=1).astype(bf16)  # [128, K*128]

    iota4 = np.tile(np.arange(P, dtype=np.float32), (P, 4)).astype(bf16)
    biasrep = np.tile(bias[None, :], (P, 1)).astype(np.float32)

    x0_pieces = [
        np.concatenate([x0[i * cfg.VSLICE:i * cfg.VSLICE + cfg.S[0]]
                        for i in range(cfg.NCORE)], 0),
        np.concatenate([x0[i * cfg.VSLICE + cfg.S[0]:(i + 1) * cfg.VSLICE]
                        for i in range(cfg.NCORE)], 0),
    ]
    in_maps = []
    for ci in range(cfg.NCORE):
        pc = percore[ci]
        in_maps.append({
            "x0_p0": x0_pieces[0],
            "x0_p1": x0_pieces[1],
            "x0_own": x0[ci * cfg.VSLICE:(ci + 1) * cfg.VSLICE],
            "gidx": pc["gidx"],
            "rowrel": pc["rowrel"],
            "vals1": pc["vals1"],
            "vals2": pc["vals2"],
            "iota4": iota4,
            "wT": wT,
            "biasrep": biasrep,
        })
    return in_maps


def build_executable(nc, in_maps, n_cores):
    """Lower the bass graph to a reusable jitted PJRT callable.

    Returns (run_once, dev_args) where run_once(*dev_args) executes the NEFF
    on cores 0..n_cores-1 and returns the concatenated out arrays.
    """
    import jax
    from jax.sharding import Mesh, PartitionSpec
    from jax.experimental.shard_map import shard_map
    import concourse.bass2jax as bass2jax
    import concourse.mybir as mybir_

    bass2jax.install_neuronx_cc_hook()

    partition_name = nc.partition_id_tensor.name if nc.partition_id_tensor else None
    in_names, out_names, out_avals = [], [], []
    zero_outs = []
    for alloc in nc.m.functions[0].allocations:
        if not isinstance(alloc, mybir_.MemoryLocationSet):
            continue
        name = alloc.memorylocations[0].name
        if alloc.kind == "ExternalInput":
            if name != partition_name:
                in_names.append(name)
        elif alloc.kind == "ExternalOutput":
            out_names.append(name)
            shape = tuple(alloc.tensor_shape)
            dtype = mybir_.dt.np(alloc.dtype)
            out_avals.append(jax.core.ShapedArray(shape, dtype))
            zero_outs.append(np.zeros(shape, dtype))
    n_params = len(in_names)
    all_in_names = list(in_names) + list(out_names)
    if partition_name is not None:
        all_in_names.append(partition_name)

    def _body(*args):
        operands = list(args)
        if partition_name is not None:
            operands.append(bass2jax.partition_id_tensor())
        outs = bass2jax._bass_exec_p.bind(
            *operands,
            out_avals=tuple(out_avals),
            in_names=tuple(all_in_names),
            out_names=tuple(out_names),
            lowering_input_output_aliases=(),
            sim_require_finite=True,
            sim_require_nnan=True,
            nc=nc,
        )
        return tuple(outs)

    devices = jax.devices()[:n_cores]
    mesh = Mesh(np.asarray(devices), ("core",))
    in_specs = (PartitionSpec("core"),) * (n_params + len(out_names))
    out_specs = (PartitionSpec("core"),) * len(out_names)
    sharded = jax.jit(
        shard_map(_body, mesh=mesh, in_specs=in_specs, out_specs=out_specs,
                  check_rep=False),
        keep_unused=True,
    )
    concat_in = [
        np.concatenate([np.asarray(in_maps[c][name]) for c in range(n_cores)], axis=0)
        for name in in_names
    ]
    concat_zeros = [
        np.zeros((n_cores * z.shape[0], *z.shape[1:]), z.dtype) for z in zero_outs
    ]
    sharding = jax.sharding.NamedSharding(mesh, PartitionSpec("core"))
    dev_args = [jax.device_put(a, sharding) for a in concat_in + concat_zeros]
    return sharded, dev_args, out_names, out_avals


def prepare(lap_rows, lap_cols, lap_vals, inputs, weight, bias, *, cfg=None):
    """Preprocess + build + lower. Returns (run, assemble)."""
    cfg = cfg or Cfg()
    rows = np.asarray(lap_rows).astype(np.int64)
    cols = np.asarray(lap_cols).astype(np.int64)
    vals = np.asarray(lap_vals).astype(np.float32)
    inputs = np.asarray(inputs, dtype=np.float32)
    weight = np.asarray(weight, dtype=np.float32)
    bias = np.asarray(bias, dtype=np.float32)

    schedule, ntt, percore = preprocess(cfg, rows, cols, vals)
    nc = build_graph(cfg, schedule, ntt)
    if not nc.is_finalized():
        nc.finalize()
    in_maps = make_inputs_maps(cfg, schedule, ntt, percore, inputs, weight, bias)
    sharded, dev_args, out_names, out_avals = build_executable(
        nc, in_maps, cfg.NCORE)

    def run():
        return sharded(*dev_args)

    def assemble(out_arrs):
        oi = out_names.index("out")
        full = np.asarray(out_arrs[oi]).reshape(
            cfg.NCORE, *out_avals[oi].shape)           # [NCORE, B, VSLICE, F]
        full = np.concatenate(list(full), axis=1)[:, :cfg.V, :]
        return full.astype(np.float32)

    return run, assemble


def kernel(lap_rows, lap_cols, lap_vals, inputs, weight, bias, *, cfg=None):
    run, assemble = prepare(lap_rows, lap_cols, lap_vals, inputs, weight, bias,
                            cfg=cfg)
    out_arrs = run()
    return assemble(out_arrs)



# revision 2
# speedup vs baseline: 1.7869x; 1.7869x over previous
"""ChebConv (K=5) distributed Trainium2 kernel over 8 NeuronCores.

Strategy: shard V across the 8 cores (row slices). Per Chebyshev step the
spmm is computed per-core for its row slice: neighbor features are fetched
with GPSIMD dma_gather (512B fp8 elements = all 4 batches x 128 feat) from a
replicated full-V DRAM copy, spread over 2 SWDGE queues; the per-edge scale +
segment-sum runs on the TensorEngine as selection matmuls (S.T @ G, S bf16
built on DVE with vals pre-scaled x64, G fp8) accumulated in PSUM per 128-row
block; an 8-way fp8 AllGather rebuilds the replicated x for the next step.
The final dense matmul (sum_k xk @ Wk) is local per core and pipelined.
"""
import os
import numpy as np
import ml_dtypes

import concourse.bass as bass
import concourse.bacc as bacc
import concourse.mybir as mybir
import concourse.tile as tile
from concourse.bass_utils import run_bass_kernel_spmd

bf16 = ml_dtypes.bfloat16
f8 = ml_dtypes.float8_e4m3
P = 128
VSCALE = 64.0   # host-side multiplier folded into S; drains divide it out
GRP = 8         # S-build tile group size
NQ = 2          # SWDGE queues for gathers


class Cfg:
    def __init__(self, V=50000, B=4, FIN=128, FOUT=128, K=5, NCORE=8):
        self.V, self.B, self.FIN, self.FOUT, self.K, self.NCORE = V, B, FIN, FOUT, K, NCORE
        self.C = B * FIN                      # 512 feature columns
        vpad = -(-V // (NCORE * P)) * (NCORE * P)
        self.VPAD = vpad                      # 50176
        self.VSLICE = vpad // NCORE           # 6272
        self.NBLK = self.VSLICE // P          # 49 row blocks per core
        # pieces: split each core's shard rows into two block-aligned pieces.
        # Piece buffers concat the piece across cores -> gather idx stays
        # within int16 (max NCORE*S1).
        self.PBLK = [self.NBLK // 2, self.NBLK - self.NBLK // 2]
        self.S = [self.PBLK[0] * P, self.PBLK[1] * P]      # rows per piece
        self.PSZ = [self.NCORE * self.S[0], self.NCORE * self.S[1]]

    def piece_idx(self, v):
        """Map node ids to (piece, index-in-piece-buffer). v: np.ndarray."""
        core = v // self.VSLICE
        r = v % self.VSLICE
        p = (r >= self.S[0]).astype(np.int64)
        idx = np.where(p == 0, core * self.S[0] + r,
                       core * self.S[1] + (r - self.S[0]))
        return p, idx


def preprocess(cfg, rows, cols, vals):
    """Sort edges by row, build the (uniform-across-cores) tile schedule and
    per-core packed gather/S-build arrays."""
    order = np.argsort(rows, kind="stable")
    r, c, v = rows[order], cols[order], vals[order]
    percore_groups = []
    for ci in range(cfg.NCORE):
        r0, r1 = ci * cfg.VSLICE, (ci + 1) * cfg.VSLICE
        lo_i, hi_i = np.searchsorted(r, [r0, r1])
        rc, cc, vc = r[lo_i:hi_i], c[lo_i:hi_i], v[lo_i:hi_i]
        blk = (rc - r0) // P
        bstart = np.searchsorted(blk, np.arange(cfg.NBLK))
        bend = np.searchsorted(blk, np.arange(cfg.NBLK) + 1)
        cp, cidx = cfg.piece_idx(cc)
        groups = []
        for b in range(cfg.NBLK):
            sl = slice(bstart[b], bend[b])
            eb_p, eb_i = cp[sl], cidx[sl]
            eb_r = rc[sl] - r0 - b * P
            eb_v = vc[sl]
            lo = eb_p == 0
            grp = []
            for m in (lo, ~lo):
                gi, gr, gv = eb_i[m], eb_r[m], eb_v[m]
                o = np.argsort(gi, kind="stable")  # ascending addresses
                grp.append((gi[o], gr[o], gv[o]))
            groups.append(tuple(grp))
        percore_groups.append(groups)

    # uniform schedule: per (block, half) tile count = max over cores
    schedule = []
    for b in range(cfg.NBLK):
        tlo = max(max(1, -(-len(g[b][0][0]) // P)) for g in percore_groups)
        thi = max(max(1, -(-len(g[b][1][0]) // P)) for g in percore_groups)
        schedule.append((tlo, thi))

    ntt = sum(tlo + thi for tlo, thi in schedule)
    percore = []
    for ci in range(cfg.NCORE):
        gidx = np.zeros((16, ntt * 8), np.int16)
        rowrel = np.zeros((P, ntt), bf16)
        vals1 = np.zeros((P, ntt), np.float32)
        t0 = 0
        for b in range(cfg.NBLK):
            for half in (0, 1):
                hc, hr, hv = percore_groups[ci][b][half]
                T = schedule[b][half]
                n = T * P
                ci_pad = np.zeros(n, np.int16)
                ci_pad[:len(hc)] = hc.astype(np.int16)
                # wrapped layout: idx[p, s] = edge[s*16 + p]
                gidx[:, t0 * 8:(t0 + T) * 8] = ci_pad.reshape(-1, 16).T
                rr = np.zeros(n, np.float32)
                rr[:len(hr)] = hr
                vv = np.zeros(n, np.float32)
                vv[:len(hv)] = hv * VSCALE
                rowrel[:, t0:t0 + T] = rr.reshape(T, P).T.astype(bf16)
                vals1[:, t0:t0 + T] = vv.reshape(T, P).T
                t0 += T
        percore.append(dict(
            gidx=np.tile(gidx, (8, 1)),   # idx AP spans 128 partitions
            rowrel=rowrel,
            vals=vals1.astype(bf16),
        ))
    return schedule, ntt, percore


def build_graph(cfg, schedule, ntt, n_steps=None, with_coll=True, with_final=True,
                skip_gather=False, skip_mm=False):
    """Build the SPMD bass graph (identical for all cores)."""
    nc = bacc.Bacc(num_swdge_queues=NQ)
    f32, bf, i16 = mybir.dt.float32, mybir.dt.bfloat16, mybir.dt.int16
    fp8 = mybir.dt.float8e4
    C, K, NBLK, VSLICE = cfg.C, cfg.K, cfg.NBLK, cfg.VSLICE

    # ---- parameters -----------------------------------------------------
    x0_p0 = nc.declare_dram_parameter("x0_p0", [cfg.PSZ[0], C], fp8, isOutput=False)
    x0_p1 = nc.declare_dram_parameter("x0_p1", [cfg.PSZ[1], C], fp8, isOutput=False)
    x0_own = nc.declare_dram_parameter("x0_own", [VSLICE, C], bf, isOutput=False)
    gidx_p = nc.declare_dram_parameter("gidx", [P, ntt * 8], i16, isOutput=False)
    rowrel_p = nc.declare_dram_parameter("rowrel", [P, ntt], bf, isOutput=False)
    vals_p = nc.declare_dram_parameter("vals", [P, ntt], bf, isOutput=False)
    iota_p = nc.declare_dram_parameter("iota8", [P, GRP * P], bf, isOutput=False)
    wT_p = nc.declare_dram_parameter("wT", [P, K * P], bf, isOutput=False)
    biasrep_p = nc.declare_dram_parameter("biasrep", [P, P], f32, isOutput=False)
    out_p = nc.declare_dram_parameter("out", [cfg.B, VSLICE, cfg.FOUT], f32, isOutput=True)

    # ---- internal DRAM --------------------------------------------------
    xk_own = [None] + [nc.dram_tensor(f"xk_own{k}", [VSLICE, C], bf) for k in range(1, K)]
    xk8_own = [None] + [nc.dram_tensor(f"xk8_own{k}", [VSLICE, C], fp8)
                        for k in range(1, K - 1)]
    xp = [None] + [
        [nc.dram_tensor(f"xp{s}_{pc}", [cfg.PSZ[pc], C], fp8, addr_space="Shared")
         for pc in range(2)]
        for s in range(1, K - 1)
    ]

    replica_groups = [list(range(cfg.NCORE))]

    with tile.TileContext(nc) as tc:
        with (
            tc.tile_pool(name="const", bufs=1) as constp,
            tc.tile_pool(name="gbuf", bufs=2 * NQ) as gbufp,
            tc.tile_pool(name="ltp", bufs=2) as ltp,
            tc.tile_pool(name="sbuf", bufs=4) as sbufp,
            tc.tile_pool(name="spool", bufs=6) as spoolp,
            tc.tile_pool(name="psum", bufs=3, space="PSUM") as psump,
            tc.tile_pool(name="psum2", bufs=2, space="PSUM") as psum2p,
        ):
            # resident constants
            gidx_t = constp.tile([P, ntt * 8], i16)
            nc.sync.dma_start(out=gidx_t[:], in_=gidx_p[:])
            rowrel_t = constp.tile([P, ntt], bf)
            nc.sync.dma_start(out=rowrel_t[:], in_=rowrel_p[:])
            vals_t = constp.tile([P, ntt], bf)
            nc.sync.dma_start(out=vals_t[:], in_=vals_p[:])
            iota_t = constp.tile([P, GRP * P], bf)
            nc.sync.dma_start(out=iota_t[:], in_=iota_p[:])
            wT_t = constp.tile([P, K * P], bf)
            nc.sync.dma_start(out=wT_t[:], in_=wT_p[:])
            biasrep_t = constp.tile([P, P], f32)
            nc.sync.dma_start(out=biasrep_t[:], in_=biasrep_p[:])

            maxT = max(tlo + thi for tlo, thi in schedule)

            # ---- Chebyshev recurrence: spmm steps s=0..K-2 --------------
            for s in range(K - 1 if n_steps is None else n_steps):
                if s == 0:
                    src_lo, src_hi = x0_p0[:], x0_p1[:]
                else:
                    src_lo, src_hi = xp[s][0][:], xp[s][1][:]
                t0 = 0
                for b in range(NBLK):
                    tlo, thi = schedule[b]
                    T = tlo + thi
                    g_t = gbufp.tile([P, maxT, C], fp8, tag="g")
                    if skip_gather:
                        nc.vector.memset(g_t[:], 0.0)
                    else:
                        for half, (toff, tcnt) in enumerate(((0, tlo), (tlo, thi))):
                            src = src_hi if half else src_lo
                            n = tcnt * P
                            nc.gpsimd.dma_gather(
                                out_ap=g_t[:, toff:toff + tcnt, :],
                                in_ap=src,
                                idxs_ap=gidx_t[:, (t0 + toff) * 8:(t0 + toff + tcnt) * 8],
                                num_idxs=n,
                                num_idxs_reg=n,
                                elem_size=C,
                                single_packet=False,
                                queue_num=half % NQ,
                            )
                    if skip_mm:
                        t0 += T
                        continue
                    psum_t = psump.tile([P, C], f32, tag="ps")
                    # selection matmuls: build S in groups of <=GRP tiles
                    for g0 in range(0, T, GRP):
                        gw = min(GRP, T - g0)
                        s_t = spoolp.tile([P, GRP * P], bf, tag="s")
                        rr = rowrel_t[:, t0 + g0:t0 + g0 + gw]
                        nc.vector.tensor_tensor(
                            out=s_t[:, :gw * P],
                            in0=rr.to_broadcast([P, gw, P]),
                            in1=iota_t[:, :gw * P],
                            op=mybir.AluOpType.is_equal,
                        )
                        vv = vals_t[:, t0 + g0:t0 + g0 + gw]
                        nc.vector.tensor_tensor(
                            out=s_t[:, :gw * P],
                            in0=s_t[:, :gw * P].rearrange("p (g q) -> p g q", q=P),
                            in1=vv.to_broadcast([P, gw, P]),
                            op=mybir.AluOpType.mult,
                        )
                        for ti in range(gw):
                            t = g0 + ti
                            nc.tensor.matmul(
                                psum_t[:],
                                lhsT=s_t[:, ti * P:(ti + 1) * P],
                                rhs=g_t[:, t, :],
                                start=(t == 0),
                                stop=(t == T - 1),
                            )
                    # drain: x_{s+1} = psum/VS (s==0) or psum*(2/VS) - x_{s-1}
                    xk_t = sbufp.tile([P, C], bf, tag="xk")
                    if s == 0:
                        nc.scalar.activation(
                            xk_t[:], psum_t[:],
                            mybir.ActivationFunctionType.Identity,
                            scale=1.0 / VSCALE)
                    else:
                        xprev_src = x0_own if s == 1 else xk_own[s - 1]
                        xprev_t = sbufp.tile([P, C], bf, tag="xprev")
                        nc.sync.dma_start(
                            out=xprev_t[:], in_=xprev_src[b * P:(b + 1) * P, :])
                        nc.vector.scalar_tensor_tensor(
                            out=xk_t[:],
                            in0=psum_t[:],
                            scalar=2.0 / VSCALE,
                            in1=xprev_t[:],
                            op0=mybir.AluOpType.mult,
                            op1=mybir.AluOpType.subtract,
                        )
                    nc.sync.dma_start(
                        out=xk_own[s + 1][b * P:(b + 1) * P, :], in_=xk_t[:])
                    if s < K - 2:
                        xk8_t = sbufp.tile([P, C], fp8, tag="xk8")
                        nc.scalar.copy(out=xk8_t[:], in_=xk_t[:])
                        nc.sync.dma_start(
                            out=xk8_own[s + 1][b * P:(b + 1) * P, :], in_=xk8_t[:])
                    t0 += T
                    # replicate finished pieces of x_{s+1} for the next step
                    if s < K - 2 and with_coll:
                        if b == cfg.PBLK[0] - 1:
                            nc.gpsimd.collective_compute(
                                "AllGather", mybir.AluOpType.bypass,
                                replica_groups=replica_groups,
                                ins=[xk8_own[s + 1][:cfg.S[0], :]],
                                outs=[xp[s + 1][0][:]],
                            )
                        elif b == NBLK - 1:
                            nc.gpsimd.collective_compute(
                                "AllGather", mybir.AluOpType.bypass,
                                replica_groups=replica_groups,
                                ins=[xk8_own[s + 1][cfg.S[0]:, :]],
                                outs=[xp[s + 1][1][:]],
                            )

            # ---- final dense matmul: out[b] = sum_k xk @ Weff_k + bias --
            # bulk-transpose xk batch column blocks in quarter-slices of the
            # row range, then K matmuls per 128-row tile; pipelined via ltp.
            nq4 = 4
            qb = -(-NBLK // nq4)
            quarters = [(i * qb, min(NBLK, (i + 1) * qb)) for i in range(nq4)]
            for bb in range(cfg.B if with_final else 0):
                for hb0, hb1 in quarters:
                    if hb1 <= hb0:
                        continue
                    r0, r1 = hb0 * P, hb1 * P
                    lts = []
                    for k in range(K):
                        src = x0_own if k == 0 else xk_own[k]
                        lt = ltp.tile([P, qb * P], bf, tag=f"lt{k}")
                        nc.sync.dma_start(
                            out=lt[:, :r1 - r0],
                            in_=src[r0:r1, bb * P:(bb + 1) * P],
                            transpose=True,
                        )
                        lts.append(lt)
                    for rt in range(hb0, hb1):
                        ro = (rt - hb0) * P
                        po = psum2p.tile([P, P], f32, tag="po")
                        for k in range(K):
                            nc.tensor.matmul(
                                po[:], lhsT=lts[k][:, ro:ro + P],
                                rhs=wT_t[:, k * P:(k + 1) * P],
                                start=(k == 0), stop=(k == K - 1),
                            )
                        ot = sbufp.tile([P, P], f32, tag="ot")
                        nc.vector.tensor_tensor(
                            out=ot[:], in0=po[:], in1=biasrep_t[:],
                            op=mybir.AluOpType.add,
                        )
                        nc.sync.dma_start(
                            out=out_p[bb, rt * P:(rt + 1) * P, :], in_=ot[:])
    return nc


def make_inputs_maps(cfg, schedule, ntt, percore, inputs, weight, bias):
    """Build per-core in_maps (host-side sharding + packing)."""
    V, C, K = cfg.V, cfg.C, cfg.K
    x0 = np.zeros((cfg.VPAD, C), np.float32)
    x0[:V] = np.transpose(inputs, (1, 0, 2)).reshape(V, C)
    x0_bf = x0.astype(bf16)
    x0_f8 = x0.astype(f8)

    # reference pairs xk[..., f*K+k] with weight.reshape(Fin*K, Fout)[f*K+k]
    wr = weight.reshape(K * cfg.FIN, cfg.FOUT)
    weff = np.stack([wr[np.arange(cfg.FIN) * K + k] for k in range(K)], 0)
    wT = np.concatenate([weff[k] for k in range(K)], axis=1).astype(bf16)

    iota = np.tile(np.arange(P, dtype=np.float32), (P, GRP)).astype(bf16)
    biasrep = np.tile(bias[None, :], (P, 1)).astype(np.float32)

    x0_pieces = [
        np.concatenate([x0_f8[i * cfg.VSLICE:i * cfg.VSLICE + cfg.S[0]]
                        for i in range(cfg.NCORE)], 0),
        np.concatenate([x0_f8[i * cfg.VSLICE + cfg.S[0]:(i + 1) * cfg.VSLICE]
                        for i in range(cfg.NCORE)], 0),
    ]
    in_maps = []
    for ci in range(cfg.NCORE):
        pc = percore[ci]
        in_maps.append({
            "x0_p0": x0_pieces[0],
            "x0_p1": x0_pieces[1],
            "x0_own": x0_bf[ci * cfg.VSLICE:(ci + 1) * cfg.VSLICE],
            "gidx": pc["gidx"],
            "rowrel": pc["rowrel"],
            "vals": pc["vals"],
            "iota8": iota,
            "wT": wT,
            "biasrep": biasrep,
        })
    return in_maps


def build_executable(nc, in_maps, n_cores):
    """Lower the bass graph to a reusable jitted PJRT callable."""
    import jax
    from jax.sharding import Mesh, PartitionSpec
    from jax.experimental.shard_map import shard_map
    import concourse.bass2jax as bass2jax
    import concourse.mybir as mybir_

    bass2jax.install_neuronx_cc_hook()

    partition_name = nc.partition_id_tensor.name if nc.partition_id_tensor else None
    in_names, out_names, out_avals = [], [], []
    zero_outs = []
    for alloc in nc.m.functions[0].allocations:
        if not isinstance(alloc, mybir_.MemoryLocationSet):
            continue
        name = alloc.memorylocations[0].name
        if alloc.kind == "ExternalInput":
            if name != partition_name:
                in_names.append(name)
        elif alloc.kind == "ExternalOutput":
            out_names.append(name)
            shape = tuple(alloc.tensor_shape)
            dtype = mybir_.dt.np(alloc.dtype)
            out_avals.append(jax.core.ShapedArray(shape, dtype))
            zero_outs.append(np.zeros(shape, dtype))
    n_params = len(in_names)
    all_in_names = list(in_names) + list(out_names)
    if partition_name is not None:
        all_in_names.append(partition_name)

    def _body(*args):
        operands = list(args)
        if partition_name is not None:
            operands.append(bass2jax.partition_id_tensor())
        outs = bass2jax._bass_exec_p.bind(
            *operands,
            out_avals=tuple(out_avals),
            in_names=tuple(all_in_names),
            out_names=tuple(out_names),
            lowering_input_output_aliases=(),
            sim_require_finite=True,
            sim_require_nnan=True,
            nc=nc,
        )
        return tuple(outs)

    devices = jax.devices()[:n_cores]
    mesh = Mesh(np.asarray(devices), ("core",))
    in_specs = (PartitionSpec("core"),) * (n_params + len(out_names))
    out_specs = (PartitionSpec("core"),) * len(out_names)
    sharded = jax.jit(
        shard_map(_body, mesh=mesh, in_specs=in_specs, out_specs=out_specs,
                  check_rep=False),
        keep_unused=True,
    )
    concat_in = [
        np.concatenate([np.asarray(in_maps[c][name]) for c in range(n_cores)], axis=0)
        for name in in_names
    ]
    concat_zeros = [
        np.zeros((n_cores * z.shape[0], *z.shape[1:]), z.dtype) for z in zero_outs
    ]
    sharding = jax.sharding.NamedSharding(mesh, PartitionSpec("core"))
    dev_args = [jax.device_put(a, sharding) for a in concat_in + concat_zeros]
    return sharded, dev_args, out_names, out_avals


def prepare(lap_rows, lap_cols, lap_vals, inputs, weight, bias, *, cfg=None):
    """Preprocess + build + lower. Returns (run, assemble)."""
    cfg = cfg or Cfg()
    rows = np.asarray(lap_rows).astype(np.int64)
    cols = np.asarray(lap_cols).astype(np.int64)
    vals = np.asarray(lap_vals).astype(np.float32)
    inputs = np.asarray(inputs, dtype=np.float32)
    weight = np.asarray(weight, dtype=np.float32)
    bias = np.asarray(bias, dtype=np.float32)

    schedule, ntt, percore = preprocess(cfg, rows, cols, vals)
    nc = build_graph(cfg, schedule, ntt)
    if not nc.is_finalized():
        nc.finalize()
    in_maps = make_inputs_maps(cfg, schedule, ntt, percore, inputs, weight, bias)
    sharded, dev_args, out_names, out_avals = build_executable(
        nc, in_maps, cfg.NCORE)

    def run():
        return sharded(*dev_args)

    def assemble(out_arrs):
        oi = out_names.index("out")
        full = np.asarray(out_arrs[oi]).reshape(
            cfg.NCORE, *out_avals[oi].shape)           # [NCORE, B, VSLICE, F]
        full = np.concatenate(list(full), axis=1)[:, :cfg.V, :]
        return full.astype(np.float32)

    return run, assemble


def kernel(lap_rows, lap_cols, lap_vals, inputs, weight, bias, *, cfg=None):
    run, assemble = prepare(lap_rows, lap_cols, lap_vals, inputs, weight, bias,
                            cfg=cfg)
    out_arrs = run()
    return assemble(out_arrs)


# revision 9
# speedup vs baseline: 10.7526x; 6.0176x over previous
"""ChebConv (K=5) distributed Trainium2 kernel over 8 NeuronCores.

Strategy: shard V across the 8 cores (row slices). Per Chebyshev step the
spmm is computed per-core for its row slice: neighbor features are fetched
with GPSIMD dma_gather (512B fp8 elements = all 4 batches x 128 feat) from a
replicated full-V DRAM copy, spread over 2 SWDGE queues; the per-edge scale +
segment-sum runs on the TensorEngine as selection matmuls (S.T @ G, S bf16
built on DVE with vals pre-scaled x64, G fp8) accumulated in PSUM per 128-row
block; an 8-way fp8 AllGather rebuilds the replicated x for the next step.
The final dense matmul (sum_k xk @ Wk) is local per core and pipelined.
"""
import os
import numpy as np
import ml_dtypes

import concourse.bass as bass
import concourse.bacc as bacc
import concourse.mybir as mybir
import concourse.tile as tile
from concourse.bass_utils import run_bass_kernel_spmd

bf16 = ml_dtypes.bfloat16
f8 = ml_dtypes.float8_e4m3
P = 128
VSCALE = 64.0   # host-side multiplier folded into S; drains divide it out
GRP = 8         # S-build tile group size
NQ = 2          # SWDGE queues for gathers


class Cfg:
    def __init__(self, V=50000, B=4, FIN=128, FOUT=128, K=5, NCORE=8):
        self.V, self.B, self.FIN, self.FOUT, self.K, self.NCORE = V, B, FIN, FOUT, K, NCORE
        self.C = B * FIN                      # 512 feature columns
        vpad = -(-V // (NCORE * P)) * (NCORE * P)
        self.VPAD = vpad                      # 50176
        self.VSLICE = vpad // NCORE           # 6272
        self.NBLK = self.VSLICE // P          # 49 row blocks per core
        # pieces: split each core's shard rows into two block-aligned pieces.
        # Piece buffers concat the piece across cores -> gather idx stays
        # within int16 (max NCORE*S1).
        self.PBLK = [self.NBLK // 2, self.NBLK - self.NBLK // 2]
        self.S = [self.PBLK[0] * P, self.PBLK[1] * P]      # rows per piece
        self.PSZ = [self.NCORE * self.S[0], self.NCORE * self.S[1]]

    def piece_idx(self, v):
        """Map node ids to (piece, index-in-piece-buffer). v: np.ndarray."""
        core = v // self.VSLICE
        r = v % self.VSLICE
        p = (r >= self.S[0]).astype(np.int64)
        idx = np.where(p == 0, core * self.S[0] + r,
                       core * self.S[1] + (r - self.S[0]))
        return p, idx


def preprocess(cfg, rows, cols, vals):
    """Sort edges by row, build the (uniform-across-cores) tile schedule and
    per-core packed gather/S-build arrays."""
    order = np.argsort(rows, kind="stable")
    r, c, v = rows[order], cols[order], vals[order]
    percore_groups = []
    for ci in range(cfg.NCORE):
        r0, r1 = ci * cfg.VSLICE, (ci + 1) * cfg.VSLICE
        lo_i, hi_i = np.searchsorted(r, [r0, r1])
        rc, cc, vc = r[lo_i:hi_i], c[lo_i:hi_i], v[lo_i:hi_i]
        blk = (rc - r0) // P
        bstart = np.searchsorted(blk, np.arange(cfg.NBLK))
        bend = np.searchsorted(blk, np.arange(cfg.NBLK) + 1)
        cp, cidx = cfg.piece_idx(cc)
        groups = []
        for b in range(cfg.NBLK):
            sl = slice(bstart[b], bend[b])
            eb_p, eb_i = cp[sl], cidx[sl]
            eb_r = rc[sl] - r0 - b * P
            eb_v = vc[sl]
            lo = eb_p == 0
            grp = []
            for m in (lo, ~lo):
                gi, gr, gv = eb_i[m], eb_r[m], eb_v[m]
                o = np.argsort(gi, kind="stable")  # ascending addresses
                grp.append((gi[o], gr[o], gv[o]))
            groups.append(tuple(grp))
        percore_groups.append(groups)

    # uniform schedule: per (block, half) tile count = max over cores
    schedule = []
    for b in range(cfg.NBLK):
        tlo = max(max(1, -(-len(g[b][0][0]) // P)) for g in percore_groups)
        thi = max(max(1, -(-len(g[b][1][0]) // P)) for g in percore_groups)
        schedule.append((tlo, thi))

    ntt = sum(tlo + thi for tlo, thi in schedule)
    percore = []
    for ci in range(cfg.NCORE):
        gidx = np.zeros((16, ntt * 8), np.int16)
        rowrel = np.zeros((P, ntt), bf16)
        vals1 = np.zeros((P, ntt), np.float32)
        counts = np.zeros(2 * cfg.NBLK, np.int32)
        t0 = 0
        for b in range(cfg.NBLK):
            for half in (0, 1):
                hc, hr, hv = percore_groups[ci][b][half]
                T = schedule[b][half]
                n = T * P
                nv = max(1, len(hc))   # >=1 valid gather (HW/interp edge case)
                counts[2 * b + half] = nv
                ci_pad = np.full(n, -1, np.int16)
                ci_pad[:len(hc)] = hc.astype(np.int16)
                ci_pad[:nv][ci_pad[:nv] < 0] = 0
                # wrapped layout: idx[p, s] = edge[s*16 + p]
                gidx[:, t0 * 8:(t0 + T) * 8] = ci_pad.reshape(-1, 16).T
                rr = np.zeros(n, np.float32)
                rr[:len(hr)] = hr
                vv = np.zeros(n, np.float32)
                vv[:len(hv)] = hv * VSCALE
                rowrel[:, t0:t0 + T] = rr.reshape(T, P).T.astype(bf16)
                vals1[:, t0:t0 + T] = vv.reshape(T, P).T
                t0 += T
        percore.append(dict(
            gidx=np.tile(gidx, (8, 1)),   # idx AP spans 128 partitions
            rowrel=rowrel,
            vals=vals1.astype(bf16),
            counts=counts.reshape(1, -1),
        ))
    return schedule, ntt, percore


def build_graph(cfg, schedule, ntt, n_steps=None, with_coll=True, with_final=True,
                skip_gather=False, skip_mm=False):
    """Build the SPMD bass graph (identical for all cores)."""
    nc = bacc.Bacc(num_swdge_queues=NQ)
    f32, bf, i16 = mybir.dt.float32, mybir.dt.bfloat16, mybir.dt.int16
    fp8 = mybir.dt.float8e4
    C, K, NBLK, VSLICE = cfg.C, cfg.K, cfg.NBLK, cfg.VSLICE

    # ---- parameters -----------------------------------------------------
    x0_p0 = nc.declare_dram_parameter("x0_p0", [cfg.PSZ[0], C], fp8, isOutput=False)
    x0_p1 = nc.declare_dram_parameter("x0_p1", [cfg.PSZ[1], C], fp8, isOutput=False)
    x0_own = nc.declare_dram_parameter("x0_own", [VSLICE, C], bf, isOutput=False)
    counts_p = nc.declare_dram_parameter("counts", [1, 2 * NBLK], mybir.dt.int32,
                                         isOutput=False)
    gidx_p = nc.declare_dram_parameter("gidx", [P, ntt * 8], i16, isOutput=False)
    rowrel_p = nc.declare_dram_parameter("rowrel", [P, ntt], bf, isOutput=False)
    vals_p = nc.declare_dram_parameter("vals", [P, ntt], bf, isOutput=False)
    iota_p = nc.declare_dram_parameter("iota8", [P, GRP * P], bf, isOutput=False)
    wT_p = nc.declare_dram_parameter("wT", [P, K * P], bf, isOutput=False)
    biasrep_p = nc.declare_dram_parameter("biasrep", [P, P], f32, isOutput=False)
    out_p = nc.declare_dram_parameter("out", [cfg.B, VSLICE, cfg.FOUT], f32, isOutput=True)

    # ---- internal DRAM --------------------------------------------------
    xk_own = [None] + [nc.dram_tensor(f"xk_own{k}", [VSLICE, C], bf) for k in range(1, K)]
    xk8_own = [None] + [nc.dram_tensor(f"xk8_own{k}", [VSLICE, C], fp8)
                        for k in range(1, K - 1)]
    xp = [None] + [
        [nc.dram_tensor(f"xp{s}_{pc}", [cfg.PSZ[pc], C], fp8, addr_space="Shared")
         for pc in range(2)]
        for s in range(1, K - 1)
    ]

    replica_groups = [list(range(cfg.NCORE))]

    with tile.TileContext(nc) as tc:
        with (
            tc.tile_pool(name="const", bufs=1) as constp,
            tc.tile_pool(name="gbuf", bufs=2 * NQ) as gbufp,
            tc.tile_pool(name="ltp", bufs=2) as ltp,
            tc.tile_pool(name="sbuf", bufs=4) as sbufp,
            tc.tile_pool(name="spool", bufs=6) as spoolp,
            tc.tile_pool(name="psum", bufs=3, space="PSUM") as psump,
            tc.tile_pool(name="psum2", bufs=4, space="PSUM") as psum2p,
        ):
            # resident constants
            counts_t = constp.tile([1, 2 * NBLK], mybir.dt.int32)
            nc.sync.dma_start(out=counts_t[:], in_=counts_p[:])
            gidx_t = constp.tile([P, ntt * 8], i16)
            nc.sync.dma_start(out=gidx_t[:], in_=gidx_p[:])
            rowrel_t = constp.tile([P, ntt], bf)
            nc.sync.dma_start(out=rowrel_t[:], in_=rowrel_p[:])
            vals_t = constp.tile([P, ntt], bf)
            nc.sync.dma_start(out=vals_t[:], in_=vals_p[:])
            iota_t = constp.tile([P, GRP * P], bf)
            nc.sync.dma_start(out=iota_t[:], in_=iota_p[:])
            wT_t = constp.tile([P, K * P], bf)
            nc.sync.dma_start(out=wT_t[:], in_=wT_p[:])
            biasrep_t = constp.tile([P, P], f32)
            nc.sync.dma_start(out=biasrep_t[:], in_=biasrep_p[:])

            maxT = max(tlo + thi for tlo, thi in schedule)

            # zero-init gather buffer slots: with -1 (skipped) pad indices the
            # pad lanes keep stale slot contents; the S columns there are 0 so
            # contributions vanish, but the slots must start NaN-free.
            for _ in range(2 * NQ):
                gz = gbufp.tile([P, maxT, C], fp8, tag="g")
                nc.vector.memset(gz[:], 0.0)

            nreg = nc.gpsimd.alloc_register("gather_cnt")

            # ---- Chebyshev recurrence: spmm steps s=0..K-2 --------------
            for s in range(K - 1 if n_steps is None else n_steps):
                if s == 0:
                    src_lo, src_hi = x0_p0[:], x0_p1[:]
                else:
                    src_lo, src_hi = xp[s][0][:], xp[s][1][:]
                t0 = 0
                for b in range(NBLK):
                    tlo, thi = schedule[b]
                    T = tlo + thi
                    g_t = gbufp.tile([P, maxT, C], fp8, tag="g")
                    if skip_gather:
                        nc.vector.memset(g_t[:], 0.0)
                    else:
                        for half, (toff, tcnt) in enumerate(((0, tlo), (tlo, thi))):
                            src = src_hi if half else src_lo
                            n = tcnt * P
                            nc.gpsimd.reg_load(
                                nreg, counts_t[0:1, 2 * b + half:2 * b + half + 1])
                            nc.gpsimd.dma_gather(
                                out_ap=g_t[:, toff:toff + tcnt, :],
                                in_ap=src,
                                idxs_ap=gidx_t[:, (t0 + toff) * 8:(t0 + toff + tcnt) * 8],
                                num_idxs=n,
                                num_idxs_reg=nreg,
                                elem_size=C,
                                single_packet=False,
                                queue_num=half % NQ,
                            )
                    if skip_mm:
                        t0 += T
                        continue
                    psum_t = psump.tile([P, C], f32, tag="ps")
                    # selection matmuls: build S in groups of <=GRP tiles
                    for g0 in range(0, T, GRP):
                        gw = min(GRP, T - g0)
                        s_t = spoolp.tile([P, GRP * P], bf, tag="s")
                        rr = rowrel_t[:, t0 + g0:t0 + g0 + gw]
                        nc.vector.tensor_tensor(
                            out=s_t[:, :gw * P],
                            in0=rr.to_broadcast([P, gw, P]),
                            in1=iota_t[:, :gw * P],
                            op=mybir.AluOpType.is_equal,
                        )
                        vv = vals_t[:, t0 + g0:t0 + g0 + gw]
                        nc.vector.tensor_tensor(
                            out=s_t[:, :gw * P],
                            in0=s_t[:, :gw * P].rearrange("p (g q) -> p g q", q=P),
                            in1=vv.to_broadcast([P, gw, P]),
                            op=mybir.AluOpType.mult,
                        )
                        for ti in range(gw):
                            t = g0 + ti
                            nc.tensor.matmul(
                                psum_t[:],
                                lhsT=s_t[:, ti * P:(ti + 1) * P],
                                rhs=g_t[:, t, :],
                                start=(t == 0),
                                stop=(t == T - 1),
                            )
                    # drain: x_{s+1} = psum/VS (s==0) or psum*(2/VS) - x_{s-1}
                    xk_t = sbufp.tile([P, C], bf, tag="xk")
                    if s == 0:
                        nc.scalar.activation(
                            xk_t[:], psum_t[:],
                            mybir.ActivationFunctionType.Identity,
                            scale=1.0 / VSCALE)
                    else:
                        xprev_src = x0_own if s == 1 else xk_own[s - 1]
                        xprev_t = sbufp.tile([P, C], bf, tag="xprev")
                        nc.sync.dma_start(
                            out=xprev_t[:], in_=xprev_src[b * P:(b + 1) * P, :])
                        nc.vector.scalar_tensor_tensor(
                            out=xk_t[:],
                            in0=psum_t[:],
                            scalar=2.0 / VSCALE,
                            in1=xprev_t[:],
                            op0=mybir.AluOpType.mult,
                            op1=mybir.AluOpType.subtract,
                        )
                    nc.sync.dma_start(
                        out=xk_own[s + 1][b * P:(b + 1) * P, :], in_=xk_t[:])
                    if s < K - 2:
                        xk8_t = sbufp.tile([P, C], fp8, tag="xk8")
                        nc.scalar.copy(out=xk8_t[:], in_=xk_t[:])
                        nc.sync.dma_start(
                            out=xk8_own[s + 1][b * P:(b + 1) * P, :], in_=xk8_t[:])
                    t0 += T
                    # replicate finished pieces of x_{s+1} for the next step
                    if s < K - 2 and with_coll:
                        if b == cfg.PBLK[0] - 1:
                            nc.gpsimd.collective_compute(
                                "AllGather", mybir.AluOpType.bypass,
                                replica_groups=replica_groups,
                                ins=[xk8_own[s + 1][:cfg.S[0], :]],
                                outs=[xp[s + 1][0][:]],
                            )
                        elif b == NBLK - 1:
                            nc.gpsimd.collective_compute(
                                "AllGather", mybir.AluOpType.bypass,
                                replica_groups=replica_groups,
                                ins=[xk8_own[s + 1][cfg.S[0]:, :]],
                                outs=[xp[s + 1][1][:]],
                            )

            # ---- final dense matmul: out[b] = sum_k xk @ Weff_k + bias --
            # bulk-transpose xk batch column blocks in quarter-slices of the
            # row range, then K matmuls per 128-row tile. Quarter-outer order
            # so each quarter's work unblocks as step-3 drains reach it and
            # overlaps the remaining spmm.
            nq4 = 4
            qb = -(-NBLK // nq4)
            quarters = [(i * qb, min(NBLK, (i + 1) * qb)) for i in range(nq4)]
            for hb0, hb1 in (quarters if with_final else []):
                if hb1 <= hb0:
                    continue
                r0, r1 = hb0 * P, hb1 * P
                for bb in range(cfg.B):
                    lts = []
                    for k in range(K):
                        src = x0_own if k == 0 else xk_own[k]
                        lt = ltp.tile([P, qb * P], bf, tag=f"lt{k}")
                        nc.sync.dma_start(
                            out=lt[:, :r1 - r0],
                            in_=src[r0:r1, bb * P:(bb + 1) * P],
                            transpose=True,
                        )
                        lts.append(lt)
                    for rt in range(hb0, hb1):
                        ro = (rt - hb0) * P
                        po = psum2p.tile([P, P], f32, tag="po")
                        for k in range(K):
                            nc.tensor.matmul(
                                po[:], lhsT=lts[k][:, ro:ro + P],
                                rhs=wT_t[:, k * P:(k + 1) * P],
                                start=(k == 0), stop=(k == K - 1),
                            )
                        ot = sbufp.tile([P, P], f32, tag="ot")
                        nc.vector.tensor_tensor(
                            out=ot[:], in0=po[:], in1=biasrep_t[:],
                            op=mybir.AluOpType.add,
                        )
                        nc.sync.dma_start(
                            out=out_p[bb, rt * P:(rt + 1) * P, :], in_=ot[:])
    return nc


def make_inputs_maps(cfg, schedule, ntt, percore, inputs, weight, bias):
    """Build per-core in_maps (host-side sharding + packing)."""
    V, C, K = cfg.V, cfg.C, cfg.K
    x0 = np.zeros((cfg.VPAD, C), np.float32)
    x0[:V] = np.transpose(inputs, (1, 0, 2)).reshape(V, C)
    x0_bf = x0.astype(bf16)
    x0_f8 = x0.astype(f8)

    # reference pairs xk[..., f*K+k] with weight.reshape(Fin*K, Fout)[f*K+k]
    wr = weight.reshape(K * cfg.FIN, cfg.FOUT)
    weff = np.stack([wr[np.arange(cfg.FIN) * K + k] for k in range(K)], 0)
    wT = np.concatenate([weff[k] for k in range(K)], axis=1).astype(bf16)

    iota = np.tile(np.arange(P, dtype=np.float32), (P, GRP)).astype(bf16)
    biasrep = np.tile(bias[None, :], (P, 1)).astype(np.float32)

    x0_pieces = [
        np.concatenate([x0_f8[i * cfg.VSLICE:i * cfg.VSLICE + cfg.S[0]]
                        for i in range(cfg.NCORE)], 0),
        np.concatenate([x0_f8[i * cfg.VSLICE + cfg.S[0]:(i + 1) * cfg.VSLICE]
                        for i in range(cfg.NCORE)], 0),
    ]
    in_maps = []
    for ci in range(cfg.NCORE):
        pc = percore[ci]
        in_maps.append({
            "x0_p0": x0_pieces[0],
            "x0_p1": x0_pieces[1],
            "x0_own": x0_bf[ci * cfg.VSLICE:(ci + 1) * cfg.VSLICE],
            "counts": pc["counts"],
            "gidx": pc["gidx"],
            "rowrel": pc["rowrel"],
            "vals": pc["vals"],
            "iota8": iota,
            "wT": wT,
            "biasrep": biasrep,
        })
    return in_maps


def build_executable(nc, in_maps, n_cores):
    """Lower the bass graph to a reusable jitted PJRT callable."""
    import jax
    from jax.sharding import Mesh, PartitionSpec
    from jax.experimental.shard_map import shard_map
    import concourse.bass2jax as bass2jax
    import concourse.mybir as mybir_

    bass2jax.install_neuronx_cc_hook()

    partition_name = nc.partition_id_tensor.name if nc.partition_id_tensor else None
    in_names, out_names, out_avals = [], [], []
    zero_outs = []
    for alloc in nc.m.functions[0].allocations:
        if not isinstance(alloc, mybir_.MemoryLocationSet):
            continue
        name = alloc.memorylocations[0].name
        if alloc.kind == "ExternalInput":
            if name != partition_name:
                in_names.append(name)
        elif alloc.kind == "ExternalOutput":
            out_names.append(name)
            shape = tuple(alloc.tensor_shape)
            dtype = mybir_.dt.np(alloc.dtype)
            out_avals.append(jax.core.ShapedArray(shape, dtype))
            zero_outs.append(np.zeros(shape, dtype))
    n_params = len(in_names)
    all_in_names = list(in_names) + list(out_names)
    if partition_name is not None:
        all_in_names.append(partition_name)

    def _body(*args):
        operands = list(args)
        if partition_name is not None:
            operands.append(bass2jax.partition_id_tensor())
        outs = bass2jax._bass_exec_p.bind(
            *operands,
            out_avals=tuple(out_avals),
            in_names=tuple(all_in_names),
            out_names=tuple(out_names),
            lowering_input_output_aliases=(),
            sim_require_finite=True,
            sim_require_nnan=True,
            nc=nc,
        )
        return tuple(outs)

    devices = jax.devices()[:n_cores]
    mesh = Mesh(np.asarray(devices), ("core",))
    in_specs = (PartitionSpec("core"),) * (n_params + len(out_names))
    out_specs = (PartitionSpec("core"),) * len(out_names)
    sharded = jax.jit(
        shard_map(_body, mesh=mesh, in_specs=in_specs, out_specs=out_specs,
                  check_rep=False),
        keep_unused=True,
    )
    concat_in = [
        np.concatenate([np.asarray(in_maps[c][name]) for c in range(n_cores)], axis=0)
        for name in in_names
    ]
    concat_zeros = [
        np.zeros((n_cores * z.shape[0], *z.shape[1:]), z.dtype) for z in zero_outs
    ]
    sharding = jax.sharding.NamedSharding(mesh, PartitionSpec("core"))
    dev_args = [jax.device_put(a, sharding) for a in concat_in + concat_zeros]
    return sharded, dev_args, out_names, out_avals


def prepare(lap_rows, lap_cols, lap_vals, inputs, weight, bias, *, cfg=None):
    """Preprocess + build + lower. Returns (run, assemble)."""
    cfg = cfg or Cfg()
    rows = np.asarray(lap_rows).astype(np.int64)
    cols = np.asarray(lap_cols).astype(np.int64)
    vals = np.asarray(lap_vals).astype(np.float32)
    inputs = np.asarray(inputs, dtype=np.float32)
    weight = np.asarray(weight, dtype=np.float32)
    bias = np.asarray(bias, dtype=np.float32)

    schedule, ntt, percore = preprocess(cfg, rows, cols, vals)
    nc = build_graph(cfg, schedule, ntt)
    if not nc.is_finalized():
        nc.finalize()
    in_maps = make_inputs_maps(cfg, schedule, ntt, percore, inputs, weight, bias)
    sharded, dev_args, out_names, out_avals = build_executable(
        nc, in_maps, cfg.NCORE)

    def run():
        return sharded(*dev_args)

    def assemble(out_arrs):
        oi = out_names.index("out")
        full = np.asarray(out_arrs[oi]).reshape(
            cfg.NCORE, *out_avals[oi].shape)           # [NCORE, B, VSLICE, F]
        full = np.concatenate(list(full), axis=1)[:, :cfg.V, :]
        return full.astype(np.float32)

    return run, assemble


def kernel(lap_rows, lap_cols, lap_vals, inputs, weight, bias, *, cfg=None):
    run, assemble = prepare(lap_rows, lap_cols, lap_vals, inputs, weight, bias,
                            cfg=cfg)
    out_arrs = run()
    return assemble(out_arrs)
